# revision 13
# baseline (speedup 1.0000x reference)
"""AbstractKWTA kernel for 8 Trainium2 NeuronCores.

Model (per batch b, all in f32):
    z = weight @ x[b]                      # [N=1024, T=512], C=2048 contraction
    recurrent scan over T:
        tot  = sum_n s
        u    = 0.75*u + z_t + (1+a)*s + (RBIAS - tot)
        v    = 0.9*v + u
        s    = (v >= 1);  v = v * (1 - s)
    out[..., t+1] = s_t  (one-step delay, out[..., 0] = 0)

Sharding: data-parallel over batch B=64 -> 8 cores x 8 batches.

z matmul: split-fp32r — operands split hi/lo at 12 significant bits (fp32r
products are exact at that width); z = Xh'Wh + Xh'Wl + Xl'Wh at ~1.3 cyc/row
vs fp32's 4, fp32-class accuracy (err ~2e-7). Stationary is the x tile so z
lands in [t, n] layout, staged through DRAM.

Scan: [128 partitions = (batch, neuron-group), 64 free] tiles, one step every
~1.6us. The per-step critical cycle is kept entirely on the Vector engine:
cpred(reset mv) -> u(STT) -> v(TT) -> sge(+accumulate). The spike total
reaches u via a tiny bf16 PE matmul accumulating -tot onto an RBIAS-prefilled
PSUM column; (1+a)*s comes from ScalarE into PSUM. The voltage reset is
applied to the *decayed* copy mv = 0.9*v (exact: v*(1-s) then decay equals
decay then zero-where-spiked), keeping it off the critical path.

The z-production matmuls for t-blocks 1..3 are interleaved 6-per-step into
the scan of the preceding block, filling the PE's idle time, so only block 0
is produced up front. Rounding orders are chosen so the spike decisions
reproduce the f32 reference bit-exactly (verified: 0/33.5M mismatches).
"""

import numpy as np

import concourse.bacc as bacc
import concourse.tile as tile
from concourse import mybir
from concourse.bass_utils import run_bass_kernel_spmd
from concourse._compat import get_trn_type

dt = mybir.dt

B, C, T, N = 64, 2048, 512, 1024
NCORES = 8
BL = B // NCORES          # batches per core
RB = np.float32(1.0 * (N - 200) / N)   # 0.8046875: exact in f32 and bf16
CH = 16                   # scan steps per chunk; also matmul k-tiles per chunk
KC = C // 128             # 16 contraction tiles
NTCB = T // 128           # 4 t-blocks

_cache = {}


def _build(a1_val: float):
    nc = bacc.Bacc(get_trn_type() or "TRN2", target_bir_lowering=False,
                   debug=False, num_devices=NCORES)
    xh_in = nc.dram_tensor("xh", [BL, C, T], dt.float32r, kind="ExternalInput").ap()
    xl_in = nc.dram_tensor("xl", [BL, C, T], dt.float32r, kind="ExternalInput").ap()
    wh_in = nc.dram_tensor("wh", [C, N], dt.float32r, kind="ExternalInput").ap()
    wl_in = nc.dram_tensor("wl", [C, N], dt.float32r, kind="ExternalInput").ap()
    mneg_in = nc.dram_tensor("mneg", [128, 128], dt.bfloat16, kind="ExternalInput").ap()
    rbrow_in = nc.dram_tensor("rbrow", [1, 128], dt.bfloat16, kind="ExternalInput").ap()
    sout = nc.dram_tensor("sout", [T, BL * N], dt.float32, kind="ExternalOutput").ap()

    with tile.TileContext(nc) as tc:
        with tc.tile_pool(name="const", bufs=1) as constp, \
             tc.tile_pool(name="wsb", bufs=1) as wsbp, \
             tc.tile_pool(name="xsb", bufs=2) as xsbp, \
             tc.tile_pool(name="zps", bufs=2, space="PSUM") as zpsp, \
             tc.tile_pool(name="zstg", bufs=4) as zstgp, \
             tc.tile_pool(name="zdram", bufs=1, space="DRAM") as zdramp, \
             tc.tile_pool(name="state", bufs=1) as statep, \
             tc.tile_pool(name="zch", bufs=3) as zchp, \
             tc.tile_pool(name="sch", bufs=3) as schp, \
             tc.tile_pool(name="tmp", bufs=4) as tmpp, \
             tc.tile_pool(name="cps", bufs=1, space="PSUM") as cpsp, \
             tc.tile_pool(name="aps", bufs=2, space="PSUM") as apsp:

            mneg = constp.tile([128, 128], dt.bfloat16, tag="mneg")
            nc.sync.dma_start(mneg[:], mneg_in[:])
            rbrow = constp.tile([1, 128], dt.bfloat16, tag="rbrow")
            nc.sync.dma_start(rbrow[:], rbrow_in[:])
            onec = constp.tile([1, T], dt.bfloat16, tag="onec")
            nc.vector.memset(onec[:], 1.0)
            zero64 = constp.tile([128, 64], dt.float32, tag="zero64")
            nc.vector.memset(zero64[:], 0.0)

            wh_sb = wsbp.tile([128, KC * N], dt.float32r, tag="wh")
            wl_sb = wsbp.tile([128, KC * N], dt.float32r, tag="wl")
            for k in range(KC):
                nc.sync.dma_start(wh_sb[:, k * N:(k + 1) * N],
                                  wh_in[k * 128:(k + 1) * 128, :])
                nc.sync.dma_start(wl_sb[:, k * N:(k + 1) * N],
                                  wl_in[k * 128:(k + 1) * 128, :])

            zscr = zdramp.tile([T, BL * N], dt.float32, tag="zscr")
            xh_re = xh_in.rearrange("b (k p) t -> b p k t", p=128)
            xl_re = xl_in.rearrange("b (k p) t -> b p k t", p=128)

            def z_loads(tcb, b):
                xh_sb = xsbp.tile([128, KC * 128], dt.float32r, tag="xh")
                nc.sync.dma_start(
                    xh_sb[:].rearrange("p (k t) -> p k t", t=128),
                    xh_re[b, :, :, tcb * 128:(tcb + 1) * 128])
                xl_sb = xsbp.tile([128, KC * 128], dt.float32r, tag="xl")
                nc.sync.dma_start(
                    xl_sb[:].rearrange("p (k t) -> p k t", t=128),
                    xl_re[b, :, :, tcb * 128:(tcb + 1) * 128])
                zp0 = zpsp.tile([128, 512], dt.float32, tag="zp0")
                zp1 = zpsp.tile([128, 512], dt.float32, tag="zp1")
                return xh_sb, xl_sb, (zp0, zp1)

            def z_mms(g, k):
                xh_sb, xl_sb, zp = g
                xh_k = xh_sb[:, k * 128:(k + 1) * 128]
                xl_k = xl_sb[:, k * 128:(k + 1) * 128]
                for nc2 in range(2):
                    wh_k = wh_sb[:, k * N + nc2 * 512:k * N + nc2 * 512 + 512]
                    wl_k = wl_sb[:, k * N + nc2 * 512:k * N + nc2 * 512 + 512]
                    nc.tensor.matmul(zp[nc2][:], xh_k, wh_k,
                                     start=(k == 0), stop=False,
                                     skip_group_check=True)
                    nc.tensor.matmul(zp[nc2][:], xh_k, wl_k,
                                     start=False, stop=False,
                                     skip_group_check=True)
                    nc.tensor.matmul(zp[nc2][:], xl_k, wh_k,
                                     start=False, stop=(k == KC - 1),
                                     skip_group_check=True)

            def z_stores(tcb, b, g):
                _, _, zp = g
                for nc2 in range(2):
                    zs = zstgp.tile([128, 512], dt.float32, tag="zs")
                    nc.scalar.copy(zs[:], zp[nc2][:])
                    nc.sync.dma_start(
                        zscr[tcb * 128:(tcb + 1) * 128,
                             b * N + nc2 * 512:b * N + nc2 * 512 + 512],
                        zs[:])

            # ---- t-block 0 (plus block 1's first group) produced up front ----
            for b in range(BL):
                g = z_loads(0, b)
                for k in range(KC):
                    z_mms(g, k)
                z_stores(0, b, g)
            g = z_loads(1, 0)
            for k in range(KC):
                z_mms(g, k)
            z_stores(1, 0, g)

            # remaining groups, one per scan chunk (one chunk of slack)
            zwork = [(tcb, b) for tcb in range(1, NTCB) for b in range(BL)][1:]

            # ---- scan state ----
            u = statep.tile([128, 64], dt.float32, tag="u")
            nc.vector.memset(u[:], 0.0)
            s_prev = statep.tile([128, 64], dt.float32, tag="sinit")
            nc.vector.memset(s_prev[:], 0.0)
            partial = statep.tile([128, 1], dt.bfloat16, tag="pinit")
            nc.vector.memset(partial[:], 0.0)
            mvp = statep.tile([128, 64], dt.float32, tag="mvinit")
            nc.vector.memset(mvp[:], 0.0)
            mu = tmpp.tile([128, 64], dt.float32, tag="mu")
            nc.scalar.mul(mu[:], u[:], 0.75)

            c_all = cpsp.tile([128, T], dt.float32, tag="call")
            nc.tensor.matmul(c_all[:], rbrow[:], onec[:], start=True, stop=False,
                             skip_group_check=True)

            zscr_re = zscr.rearrange("t (p e) -> p t e", p=128)
            sout_re = sout.rearrange("t (p e) -> p t e", p=128)
            zg_handles = {}
            if zwork:
                zg_handles[0] = z_loads(*zwork[0])
            for ci in range(T // CH):
                if ci + 1 < len(zwork):
                    zg_handles[ci + 1] = z_loads(*zwork[ci + 1])
                zg_cur = zg_handles.pop(ci, None)

                zch = zchp.tile([128, CH * 64], dt.float32)
                nc.sync.dma_start(
                    zch[:].rearrange("p (k e) -> p k e", e=64),
                    zscr_re[:, ci * CH:(ci + 1) * CH, :])
                sch = schp.tile([128, CH * 64], dt.float32)
                for k in range(CH):
                    t = ci * CH + k
                    # PE: scan matmul first, then this step's share of z MMs
                    nc.tensor.matmul(c_all[:, t:t + 1], mneg[:], partial[:],
                                     start=False, stop=(t == T - 1),
                                     skip_group_check=True)
                    if zg_cur is not None:
                        z_mms(zg_cur, k)
                    # ACT: (1+a)*s -> PSUM (feeds this step's STT)
                    sA = apsp.tile([128, 64], dt.float32, tag="sA")
                    nc.scalar.mul(sA[:], s_prev[:], float(a1_val))
                    # DVE: zero the decayed voltage where spiked (prev step)
                    nc.vector.copy_predicated(mvp[:], s_prev[:].bitcast(dt.int32),
                                              zero64[:])
                    # GPSIMD: q = mu + z_t
                    q = tmpp.tile([128, 64], dt.float32, tag="q")
                    nc.gpsimd.tensor_tensor(q[:], mu[:], zch[:, k * 64:(k + 1) * 64],
                                            mybir.AluOpType.add)
                    # DVE: u = (q + c) + sA
                    nc.vector.scalar_tensor_tensor(
                        u[:], q[:], c_all[:, t:t + 1], sA[:],
                        mybir.AluOpType.add, mybir.AluOpType.add)
                    # ACT: mu for next step
                    mu = tmpp.tile([128, 64], dt.float32, tag="mu")
                    nc.scalar.mul(mu[:], u[:], 0.75)
                    # DVE: v = mv + u   (mv = zeroed 0.9*v_prev)
                    vp = tmpp.tile([128, 64], dt.float32, tag="vp")
                    nc.vector.tensor_tensor(vp[:], mvp[:], u[:], mybir.AluOpType.add)
                    # ACT: decayed voltage for next step (reset applied next step)
                    mvp = tmpp.tile([128, 64], dt.float32, tag="mvp")
                    nc.scalar.mul(mvp[:], vp[:], 0.9)
                    # DVE: spikes + per-partition count
                    partial = tmpp.tile([128, 1], dt.bfloat16, tag="partial")
                    s_sl = sch[:, k * 64:(k + 1) * 64]
                    nc.vector.tensor_scalar(s_sl, vp[:], 1.0, 0.0,
                                            mybir.AluOpType.is_ge,
                                            mybir.AluOpType.add,
                                            accum_out=partial[:])
                    s_prev = s_sl
                if zg_cur is not None:
                    z_stores(*zwork[ci], zg_cur)
                nc.sync.dma_start(
                    sout_re[:, ci * CH:(ci + 1) * CH, :],
                    sch[:].rearrange("p (k e) -> p k e", e=64))
    nc.compile()
    return nc


def _round_mant(a, keep=11):
    """Round f32 to `keep` explicit mantissa bits (round-to-nearest)."""
    bits = a.view(np.uint32)
    shift = 23 - keep
    half = np.uint32(1 << (shift - 1))
    mask = np.uint32((0xFFFFFFFF << shift) & 0xFFFFFFFF)
    return ((bits + half) & mask).view(np.float32)


def _split(a):
    hi = _round_mant(np.ascontiguousarray(a, np.float32))
    lo = _round_mant((a.astype(np.float64) - hi).astype(np.float32))
    return hi, lo


def kernel(x, weight, self_excitation):
    import ml_dtypes
    x = np.asarray(x, dtype=np.float32)
    weight = np.asarray(weight, dtype=np.float32)
    a = np.float32(np.clip(np.asarray(self_excitation, np.float32), 0.0, 1.0)[0])
    A1 = np.float32(np.float32(1.0) + a)

    key = float(A1)
    if _cache.get("key") != key:
        _cache["nc"] = _build(key)
        _cache["key"] = key
    nc = _cache["nc"]

    wT = np.ascontiguousarray(weight.T)                     # [C, N]
    wh, wl = _split(wT)
    xh, xl = _split(x)
    blk = np.repeat(np.arange(8), 16)
    mneg = np.where(blk[:, None] == blk[None, :], np.float32(-1.0),
                    np.float32(0.0)).astype(ml_dtypes.bfloat16)
    rbrow = np.full((1, 128), RB, ml_dtypes.bfloat16)

    in_maps = []
    for c in range(NCORES):
        in_maps.append({
            "xh": xh[c * BL:(c + 1) * BL],
            "xl": xl[c * BL:(c + 1) * BL],
            "wh": wh,
            "wl": wl,
            "mneg": mneg,
            "rbrow": rbrow,
        })
    global _last_in_maps
    _last_in_maps = in_maps
    res = run_bass_kernel_spmd(nc, in_maps, core_ids=list(range(NCORES)))

    out = np.zeros((B, N, T), np.float32)
    for c in range(NCORES):
        g = res.results[c]["sout"].reshape(T, BL, N)        # [t, b, n]
        out[c * BL:(c + 1) * BL, :, 1:] = g[:T - 1].transpose(1, 2, 0)
    return out


# revision 17
# speedup vs baseline: 1.0034x; 1.0034x over previous
"""AbstractKWTA kernel for 8 Trainium2 NeuronCores.

Model (per batch b, all in f32):
    z = weight @ x[b]                      # [N=1024, T=512], C=2048 contraction
    recurrent scan over T:
        tot  = sum_n s
        u    = 0.75*u + z_t + (1+a)*s + (RBIAS - tot)
        v    = 0.9*v + u
        s    = (v >= 1);  v = v * (1 - s)
    out[..., t+1] = s_t  (one-step delay, out[..., 0] = 0)

Sharding: data-parallel over batch B=64 -> 8 cores x 8 batches.

z matmul: split-fp32r — operands split hi/lo at 12 significant bits (fp32r
products are exact at that width); z = Xh'Wh + Xh'Wl + Xl'Wh at ~1.3 cyc/row
vs fp32's 4, fp32-class accuracy (err ~2e-7). Stationary is the x tile so z
lands in [t, n] layout, staged through DRAM.

Scan: [128 partitions = (batch, neuron-group), 64 free] tiles, one step every
~1.6us. The per-step critical cycle is kept entirely on the Vector engine:
cpred(reset mv) -> u(STT) -> v(TT) -> sge(+accumulate). The spike total
reaches u via a tiny bf16 PE matmul accumulating -tot onto an RBIAS-prefilled
PSUM column; (1+a)*s comes from ScalarE into PSUM. The voltage reset is
applied to the *decayed* copy mv = 0.9*v (exact: v*(1-s) then decay equals
decay then zero-where-spiked), keeping it off the critical path.

The z-production matmuls for t-blocks 1..3 are interleaved 6-per-step into
the scan of the preceding block, filling the PE's idle time, so only block 0
is produced up front. Rounding orders are chosen so the spike decisions
reproduce the f32 reference bit-exactly (verified: 0/33.5M mismatches).
"""

import numpy as np

import concourse.bacc as bacc
import concourse.tile as tile
from concourse import mybir
from concourse.bass_utils import run_bass_kernel_spmd
from concourse._compat import get_trn_type

dt = mybir.dt

B, C, T, N = 64, 2048, 512, 1024
NCORES = 8
BL = B // NCORES          # batches per core
RB = np.float32(1.0 * (N - 200) / N)   # 0.8046875: exact in f32 and bf16
CH = 16                   # scan steps per chunk; also matmul k-tiles per chunk
KC = C // 128             # 16 contraction tiles
NTCB = T // 128           # 4 t-blocks

_cache = {}


def _build(a1_val: float):
    nc = bacc.Bacc(get_trn_type() or "TRN2", target_bir_lowering=False,
                   debug=False, num_devices=NCORES)
    xh_in = nc.dram_tensor("xh", [BL, C, T], dt.float32r, kind="ExternalInput").ap()
    xl_in = nc.dram_tensor("xl", [BL, C, T], dt.float32r, kind="ExternalInput").ap()
    wh_in = nc.dram_tensor("wh", [C, N], dt.float32r, kind="ExternalInput").ap()
    wl_in = nc.dram_tensor("wl", [C, N], dt.float32r, kind="ExternalInput").ap()
    mneg_in = nc.dram_tensor("mneg", [128, 128], dt.bfloat16, kind="ExternalInput").ap()
    rbrow_in = nc.dram_tensor("rbrow", [1, 128], dt.bfloat16, kind="ExternalInput").ap()
    sout = nc.dram_tensor("sout", [T, BL * N], dt.float32, kind="ExternalOutput").ap()

    with tile.TileContext(nc) as tc:
        with tc.tile_pool(name="const", bufs=1) as constp, \
             tc.tile_pool(name="wsb", bufs=1) as wsbp, \
             tc.tile_pool(name="xsb", bufs=2) as xsbp, \
             tc.tile_pool(name="zps", bufs=2, space="PSUM") as zpsp, \
             tc.tile_pool(name="zstg", bufs=4) as zstgp, \
             tc.tile_pool(name="zdram", bufs=1, space="DRAM") as zdramp, \
             tc.tile_pool(name="state", bufs=1) as statep, \
             tc.tile_pool(name="zch", bufs=3) as zchp, \
             tc.tile_pool(name="sch", bufs=3) as schp, \
             tc.tile_pool(name="tmp", bufs=4) as tmpp, \
             tc.tile_pool(name="cps", bufs=1, space="PSUM") as cpsp, \
             tc.tile_pool(name="aps", bufs=2, space="PSUM") as apsp:

            mneg = constp.tile([128, 128], dt.bfloat16, tag="mneg")
            nc.sync.dma_start(mneg[:], mneg_in[:])
            rbrow = constp.tile([1, 128], dt.bfloat16, tag="rbrow")
            nc.sync.dma_start(rbrow[:], rbrow_in[:])
            onec = constp.tile([1, T], dt.bfloat16, tag="onec")
            nc.vector.memset(onec[:], 1.0)
            zero64 = constp.tile([128, 64], dt.float32, tag="zero64")
            nc.vector.memset(zero64[:], 0.0)

            wh_sb = wsbp.tile([128, KC * N], dt.float32r, tag="wh")
            wl_sb = wsbp.tile([128, KC * N], dt.float32r, tag="wl")
            for k in range(KC):
                nc.sync.dma_start(wh_sb[:, k * N:(k + 1) * N],
                                  wh_in[k * 128:(k + 1) * 128, :])
                nc.sync.dma_start(wl_sb[:, k * N:(k + 1) * N],
                                  wl_in[k * 128:(k + 1) * 128, :])

            zscr = zdramp.tile([T, BL * N], dt.float32, tag="zscr")
            xh_re = xh_in.rearrange("b (k p) t -> b p k t", p=128)
            xl_re = xl_in.rearrange("b (k p) t -> b p k t", p=128)

            def z_loads(tcb, b):
                xh_sb = xsbp.tile([128, KC * 128], dt.float32r, tag="xh")
                nc.sync.dma_start(
                    xh_sb[:].rearrange("p (k t) -> p k t", t=128),
                    xh_re[b, :, :, tcb * 128:(tcb + 1) * 128])
                xl_sb = xsbp.tile([128, KC * 128], dt.float32r, tag="xl")
                nc.sync.dma_start(
                    xl_sb[:].rearrange("p (k t) -> p k t", t=128),
                    xl_re[b, :, :, tcb * 128:(tcb + 1) * 128])
                zp0 = zpsp.tile([128, 512], dt.float32, tag="zp0")
                zp1 = zpsp.tile([128, 512], dt.float32, tag="zp1")
                return xh_sb, xl_sb, (zp0, zp1)

            def z_mms(g, k):
                xh_sb, xl_sb, zp = g
                xh_k = xh_sb[:, k * 128:(k + 1) * 128]
                xl_k = xl_sb[:, k * 128:(k + 1) * 128]

                def wslc(w, nc2):
                    return w[:, k * N + nc2 * 512:k * N + nc2 * 512 + 512]

                # xh stationary: 4 matmuls, then xl stationary: 2
                nc.tensor.matmul(zp[0][:], xh_k, wslc(wh_sb, 0),
                                 start=(k == 0), stop=False, skip_group_check=True)
                nc.tensor.matmul(zp[0][:], xh_k, wslc(wl_sb, 0),
                                 start=False, stop=False, skip_group_check=True)
                nc.tensor.matmul(zp[1][:], xh_k, wslc(wh_sb, 1),
                                 start=(k == 0), stop=False, skip_group_check=True)
                nc.tensor.matmul(zp[1][:], xh_k, wslc(wl_sb, 1),
                                 start=False, stop=False, skip_group_check=True)
                nc.tensor.matmul(zp[0][:], xl_k, wslc(wh_sb, 0),
                                 start=False, stop=(k == KC - 1), skip_group_check=True)
                nc.tensor.matmul(zp[1][:], xl_k, wslc(wh_sb, 1),
                                 start=False, stop=(k == KC - 1), skip_group_check=True)

            def z_stores(tcb, b, g):
                _, _, zp = g
                for nc2 in range(2):
                    zs = zstgp.tile([128, 512], dt.float32, tag="zs")
                    nc.scalar.copy(zs[:], zp[nc2][:])
                    nc.sync.dma_start(
                        zscr[tcb * 128:(tcb + 1) * 128,
                             b * N + nc2 * 512:b * N + nc2 * 512 + 512],
                        zs[:])

            # ---- t-block 0 (plus block 1's first group) produced up front,
            # with x loads prefetched one group ahead ----
            head = [(0, b) for b in range(BL)] + [(1, 0)]
            gh = z_loads(*head[0])
            for i, (tcb, b) in enumerate(head):
                g = gh
                if i + 1 < len(head):
                    gh = z_loads(*head[i + 1])
                for k in range(KC):
                    z_mms(g, k)
                z_stores(tcb, b, g)

            # remaining groups, one per scan chunk (one chunk of slack)
            zwork = [(tcb, b) for tcb in range(1, NTCB) for b in range(BL)][1:]

            # ---- scan state ----
            u = statep.tile([128, 64], dt.float32, tag="u")
            nc.vector.memset(u[:], 0.0)
            s_prev = statep.tile([128, 64], dt.float32, tag="sinit")
            nc.vector.memset(s_prev[:], 0.0)
            partial = statep.tile([128, 1], dt.bfloat16, tag="pinit")
            nc.vector.memset(partial[:], 0.0)
            mvp = statep.tile([128, 64], dt.float32, tag="mvinit")
            nc.vector.memset(mvp[:], 0.0)
            mu = tmpp.tile([128, 64], dt.float32, tag="mu")
            nc.scalar.mul(mu[:], u[:], 0.75)

            c_all = cpsp.tile([128, T], dt.float32, tag="call")
            nc.tensor.matmul(c_all[:], rbrow[:], onec[:], start=True, stop=False,
                             skip_group_check=True)

            zscr_re = zscr.rearrange("t (p e) -> p t e", p=128)
            sout_re = sout.rearrange("t (p e) -> p t e", p=128)
            zg_handles = {}
            if zwork:
                zg_handles[0] = z_loads(*zwork[0])
            for ci in range(T // CH):
                if ci + 1 < len(zwork):
                    zg_handles[ci + 1] = z_loads(*zwork[ci + 1])
                zg_cur = zg_handles.pop(ci, None)

                zch = zchp.tile([128, CH * 64], dt.float32)
                nc.sync.dma_start(
                    zch[:].rearrange("p (k e) -> p k e", e=64),
                    zscr_re[:, ci * CH:(ci + 1) * CH, :])
                sch = schp.tile([128, CH * 64], dt.float32)
                for k in range(CH):
                    t = ci * CH + k
                    # PE: scan matmul first, then this step's share of z MMs
                    nc.tensor.matmul(c_all[:, t:t + 1], mneg[:], partial[:],
                                     start=False, stop=(t == T - 1),
                                     skip_group_check=True)
                    if zg_cur is not None:
                        z_mms(zg_cur, k)
                    # ACT: (1+a)*s -> PSUM (feeds this step's STT)
                    sA = apsp.tile([128, 64], dt.float32, tag="sA")
                    nc.scalar.mul(sA[:], s_prev[:], float(a1_val))
                    # DVE: zero the decayed voltage where spiked (prev step)
                    nc.vector.copy_predicated(mvp[:], s_prev[:].bitcast(dt.int32),
                                              zero64[:])
                    # GPSIMD: q = mu + z_t
                    q = tmpp.tile([128, 64], dt.float32, tag="q")
                    nc.gpsimd.tensor_tensor(q[:], mu[:], zch[:, k * 64:(k + 1) * 64],
                                            mybir.AluOpType.add)
                    # DVE: u = (q + c) + sA
                    nc.vector.scalar_tensor_tensor(
                        u[:], q[:], c_all[:, t:t + 1], sA[:],
                        mybir.AluOpType.add, mybir.AluOpType.add)
                    # ACT: mu for next step
                    mu = tmpp.tile([128, 64], dt.float32, tag="mu")
                    nc.scalar.mul(mu[:], u[:], 0.75)
                    # DVE: v = mv + u   (mv = zeroed 0.9*v_prev)
                    vp = tmpp.tile([128, 64], dt.float32, tag="vp")
                    nc.vector.tensor_tensor(vp[:], mvp[:], u[:], mybir.AluOpType.add)
                    # ACT: decayed voltage for next step (reset applied next step)
                    mvp = tmpp.tile([128, 64], dt.float32, tag="mvp")
                    nc.scalar.mul(mvp[:], vp[:], 0.9)
                    # DVE: spikes + per-partition count
                    partial = tmpp.tile([128, 1], dt.bfloat16, tag="partial")
                    s_sl = sch[:, k * 64:(k + 1) * 64]
                    nc.vector.tensor_scalar(s_sl, vp[:], 1.0, 0.0,
                                            mybir.AluOpType.is_ge,
                                            mybir.AluOpType.add,
                                            accum_out=partial[:])
                    s_prev = s_sl
                if zg_cur is not None:
                    z_stores(*zwork[ci], zg_cur)
                nc.sync.dma_start(
                    sout_re[:, ci * CH:(ci + 1) * CH, :],
                    sch[:].rearrange("p (k e) -> p k e", e=64))
    nc.compile()
    return nc


def _round_mant(a, keep=11):
    """Round f32 to `keep` explicit mantissa bits (round-to-nearest)."""
    bits = a.view(np.uint32)
    shift = 23 - keep
    half = np.uint32(1 << (shift - 1))
    mask = np.uint32((0xFFFFFFFF << shift) & 0xFFFFFFFF)
    return ((bits + half) & mask).view(np.float32)


def _split(a):
    hi = _round_mant(np.ascontiguousarray(a, np.float32))
    lo = _round_mant((a.astype(np.float64) - hi).astype(np.float32))
    return hi, lo


def kernel(x, weight, self_excitation):
    import ml_dtypes
    x = np.asarray(x, dtype=np.float32)
    weight = np.asarray(weight, dtype=np.float32)
    a = np.float32(np.clip(np.asarray(self_excitation, np.float32), 0.0, 1.0)[0])
    A1 = np.float32(np.float32(1.0) + a)

    key = float(A1)
    if _cache.get("key") != key:
        _cache["nc"] = _build(key)
        _cache["key"] = key
    nc = _cache["nc"]

    wT = np.ascontiguousarray(weight.T)                     # [C, N]
    wh, wl = _split(wT)
    xh, xl = _split(x)
    blk = np.repeat(np.arange(8), 16)
    mneg = np.where(blk[:, None] == blk[None, :], np.float32(-1.0),
                    np.float32(0.0)).astype(ml_dtypes.bfloat16)
    rbrow = np.full((1, 128), RB, ml_dtypes.bfloat16)

    in_maps = []
    for c in range(NCORES):
        in_maps.append({
            "xh": xh[c * BL:(c + 1) * BL],
            "xl": xl[c * BL:(c + 1) * BL],
            "wh": wh,
            "wl": wl,
            "mneg": mneg,
            "rbrow": rbrow,
        })
    global _last_in_maps
    _last_in_maps = in_maps
    res = run_bass_kernel_spmd(nc, in_maps, core_ids=list(range(NCORES)))

    out = np.zeros((B, N, T), np.float32)
    for c in range(NCORES):
        g = res.results[c]["sout"].reshape(T, BL, N)        # [t, b, n]
        out[c * BL:(c + 1) * BL, :, 1:] = g[:T - 1].transpose(1, 2, 0)
    return out


# revision 19
# speedup vs baseline: 1.2147x; 1.2106x over previous
"""AbstractKWTA kernel for 8 Trainium2 NeuronCores.

Model (per batch b, all in f32):
    z = weight @ x[b]                      # [N=1024, T=512], C=2048 contraction
    recurrent scan over T:
        tot  = sum_n s
        u    = 0.75*u + z_t + (1+a)*s + (RBIAS - tot)
        v    = 0.9*v + u
        s    = (v >= 1);  v = v * (1 - s)
    out[..., t+1] = s_t  (one-step delay, out[..., 0] = 0)

Sharding: data-parallel over batch B=64 -> 8 cores x 8 batches.

z matmul: split-fp32r — operands split hi/lo at 12 significant bits (fp32r
products are exact at that width); z = Xh'Wh + Xh'Wl + Xl'Wh at ~1.3 cyc/row
vs fp32's 4, fp32-class accuracy (err ~2e-7). Stationary is the x tile so z
lands in [t, n] layout, staged through DRAM.

Scan: [128 partitions = (batch, neuron-group), 64 free] tiles, one step every
~1.6us. The per-step critical cycle is kept entirely on the Vector engine:
cpred(reset mv) -> u(STT) -> v(TT) -> sge(+accumulate). The spike total
reaches u via a tiny bf16 PE matmul accumulating -tot onto an RBIAS-prefilled
PSUM column; (1+a)*s comes from ScalarE into PSUM. The voltage reset is
applied to the *decayed* copy mv = 0.9*v (exact: v*(1-s) then decay equals
decay then zero-where-spiked), keeping it off the critical path.

The z-production matmuls for t-blocks 1..3 are interleaved 6-per-step into
the scan of the preceding block, filling the PE's idle time, so only block 0
is produced up front. Rounding orders are chosen so the spike decisions
reproduce the f32 reference bit-exactly (verified: 0/33.5M mismatches).
"""

import numpy as np

import concourse.bacc as bacc
import concourse.tile as tile
from concourse import mybir
from concourse.bass_utils import run_bass_kernel_spmd
from concourse._compat import get_trn_type

dt = mybir.dt

B, C, T, N = 64, 2048, 512, 1024
NCORES = 8
BL = B // NCORES          # batches per core
RB = np.float32(1.0 * (N - 200) / N)   # 0.8046875: exact in f32 and bf16
CH = 16                   # scan steps per chunk; also matmul k-tiles per chunk
KC = C // 128             # 16 contraction tiles
NTCB = T // 128           # 4 t-blocks

_cache = {}


def _build(a1_val: float):
    nc = bacc.Bacc(get_trn_type() or "TRN2", target_bir_lowering=False,
                   debug=False, num_devices=NCORES)
    xh_in = nc.dram_tensor("xh", [BL, C, T], dt.float32r, kind="ExternalInput").ap()
    xh16_in = nc.dram_tensor("xh16", [BL, C, T], dt.float16, kind="ExternalInput").ap()
    xl16_in = nc.dram_tensor("xl16", [BL, C, T], dt.float16, kind="ExternalInput").ap()
    wh_in = nc.dram_tensor("wh", [C, N], dt.float32r, kind="ExternalInput").ap()
    wh16_in = nc.dram_tensor("wh16", [C, N], dt.float16, kind="ExternalInput").ap()
    wl16_in = nc.dram_tensor("wl16", [C, N], dt.float16, kind="ExternalInput").ap()
    mneg_in = nc.dram_tensor("mneg", [128, 128], dt.bfloat16, kind="ExternalInput").ap()
    rbrow_in = nc.dram_tensor("rbrow", [1, 128], dt.bfloat16, kind="ExternalInput").ap()
    sout = nc.dram_tensor("sout", [T, BL * N], dt.float32, kind="ExternalOutput").ap()

    with tile.TileContext(nc) as tc:
        with tc.tile_pool(name="const", bufs=1) as constp, \
             tc.tile_pool(name="wsb", bufs=1) as wsbp, \
             tc.tile_pool(name="xsb", bufs=2) as xsbp, \
             tc.tile_pool(name="zps", bufs=2, space="PSUM") as zpsp, \
             tc.tile_pool(name="zstg", bufs=4) as zstgp, \
             tc.tile_pool(name="zdram", bufs=1, space="DRAM") as zdramp, \
             tc.tile_pool(name="state", bufs=1) as statep, \
             tc.tile_pool(name="zch", bufs=3) as zchp, \
             tc.tile_pool(name="sch", bufs=3) as schp, \
             tc.tile_pool(name="tmp", bufs=4) as tmpp, \
             tc.tile_pool(name="cps", bufs=1, space="PSUM") as cpsp, \
             tc.tile_pool(name="aps", bufs=2, space="PSUM") as apsp:

            mneg = constp.tile([128, 128], dt.bfloat16, tag="mneg")
            nc.sync.dma_start(mneg[:], mneg_in[:])
            rbrow = constp.tile([1, 128], dt.bfloat16, tag="rbrow")
            nc.sync.dma_start(rbrow[:], rbrow_in[:])
            onec = constp.tile([1, T], dt.bfloat16, tag="onec")
            nc.vector.memset(onec[:], 1.0)
            zero64 = constp.tile([128, 64], dt.float32, tag="zero64")
            nc.vector.memset(zero64[:], 0.0)

            wh_sb = wsbp.tile([128, KC * N], dt.float32r, tag="wh")
            wh16_sb = wsbp.tile([128, KC * N], dt.float16, tag="wh16")
            wl16_sb = wsbp.tile([128, KC * N], dt.float16, tag="wl16")
            for k in range(KC):
                nc.sync.dma_start(wh_sb[:, k * N:(k + 1) * N],
                                  wh_in[k * 128:(k + 1) * 128, :])
                nc.sync.dma_start(wh16_sb[:, k * N:(k + 1) * N],
                                  wh16_in[k * 128:(k + 1) * 128, :])
                nc.sync.dma_start(wl16_sb[:, k * N:(k + 1) * N],
                                  wl16_in[k * 128:(k + 1) * 128, :])

            zscr = zdramp.tile([T, BL * N], dt.float32, tag="zscr")
            xh_re = xh_in.rearrange("b (k p) t -> b p k t", p=128)
            xh16_re = xh16_in.rearrange("b (k p) t -> b p k t", p=128)
            xl16_re = xl16_in.rearrange("b (k p) t -> b p k t", p=128)

            def z_loads(tcb, b):
                xh_sb = xsbp.tile([128, KC * 128], dt.float32r, tag="xh")
                nc.sync.dma_start(
                    xh_sb[:].rearrange("p (k t) -> p k t", t=128),
                    xh_re[b, :, :, tcb * 128:(tcb + 1) * 128])
                xh16_sb = xsbp.tile([128, KC * 128], dt.float16, tag="xh16")
                nc.sync.dma_start(
                    xh16_sb[:].rearrange("p (k t) -> p k t", t=128),
                    xh16_re[b, :, :, tcb * 128:(tcb + 1) * 128])
                xl16_sb = xsbp.tile([128, KC * 128], dt.float16, tag="xl16")
                nc.sync.dma_start(
                    xl16_sb[:].rearrange("p (k t) -> p k t", t=128),
                    xl16_re[b, :, :, tcb * 128:(tcb + 1) * 128])
                zp0 = zpsp.tile([128, 512], dt.float32, tag="zp0")
                zp1 = zpsp.tile([128, 512], dt.float32, tag="zp1")
                return (xh_sb, xh16_sb, xl16_sb), (zp0, zp1)

            def z_mms(g, k):
                (xh_sb, xh16_sb, xl16_sb), zp = g
                xh_k = xh_sb[:, k * 128:(k + 1) * 128]
                xh16_k = xh16_sb[:, k * 128:(k + 1) * 128]
                xl16_k = xl16_sb[:, k * 128:(k + 1) * 128]

                def wslc(w, nc2):
                    return w[:, k * N + nc2 * 512:k * N + nc2 * 512 + 512]

                # each stationary serves both n-halves consecutively
                nc.tensor.matmul(zp[0][:], xh_k, wslc(wh_sb, 0),
                                 start=(k == 0), stop=False, skip_group_check=True)
                nc.tensor.matmul(zp[1][:], xh_k, wslc(wh_sb, 1),
                                 start=(k == 0), stop=False, skip_group_check=True)
                nc.tensor.matmul(zp[0][:], xh16_k, wslc(wl16_sb, 0),
                                 start=False, stop=False, skip_group_check=True)
                nc.tensor.matmul(zp[1][:], xh16_k, wslc(wl16_sb, 1),
                                 start=False, stop=False, skip_group_check=True)
                nc.tensor.matmul(zp[0][:], xl16_k, wslc(wh16_sb, 0),
                                 start=False, stop=(k == KC - 1), skip_group_check=True)
                nc.tensor.matmul(zp[1][:], xl16_k, wslc(wh16_sb, 1),
                                 start=False, stop=(k == KC - 1), skip_group_check=True)

            def z_stores(tcb, b, g):
                _, zp = g
                for nc2 in range(2):
                    zs = zstgp.tile([128, 512], dt.float32, tag="zs")
                    nc.scalar.copy(zs[:], zp[nc2][:])
                    nc.sync.dma_start(
                        zscr[tcb * 128:(tcb + 1) * 128,
                             b * N + nc2 * 512:b * N + nc2 * 512 + 512],
                        zs[:])

            # ---- t-block 0 (plus block 1's first group) produced up front,
            # with x loads prefetched one group ahead ----
            head = [(0, b) for b in range(BL)] + [(1, 0)]
            gh = z_loads(*head[0])
            for i, (tcb, b) in enumerate(head):
                g = gh
                if i + 1 < len(head):
                    gh = z_loads(*head[i + 1])
                for k in range(KC):
                    z_mms(g, k)
                z_stores(tcb, b, g)

            # remaining groups, one per scan chunk (one chunk of slack)
            zwork = [(tcb, b) for tcb in range(1, NTCB) for b in range(BL)][1:]

            # ---- scan state ----
            u = statep.tile([128, 64], dt.float32, tag="u")
            nc.vector.memset(u[:], 0.0)
            s_prev = statep.tile([128, 64], dt.float32, tag="sinit")
            nc.vector.memset(s_prev[:], 0.0)
            partial = statep.tile([128, 1], dt.bfloat16, tag="pinit")
            nc.vector.memset(partial[:], 0.0)
            mvp = statep.tile([128, 64], dt.float32, tag="mvinit")
            nc.vector.memset(mvp[:], 0.0)
            mu = tmpp.tile([128, 64], dt.float32, tag="mu")
            nc.scalar.mul(mu[:], u[:], 0.75)

            c_all = cpsp.tile([128, T], dt.float32, tag="call")
            nc.tensor.matmul(c_all[:], rbrow[:], onec[:], start=True, stop=False,
                             skip_group_check=True)

            zscr_re = zscr.rearrange("t (p e) -> p t e", p=128)
            sout_re = sout.rearrange("t (p e) -> p t e", p=128)
            zg_handles = {}
            if zwork:
                zg_handles[0] = z_loads(*zwork[0])
            for ci in range(T // CH):
                if ci + 1 < len(zwork):
                    zg_handles[ci + 1] = z_loads(*zwork[ci + 1])
                zg_cur = zg_handles.pop(ci, None)

                zch = zchp.tile([128, CH * 64], dt.float32)
                nc.sync.dma_start(
                    zch[:].rearrange("p (k e) -> p k e", e=64),
                    zscr_re[:, ci * CH:(ci + 1) * CH, :])
                sch = schp.tile([128, CH * 64], dt.float32)
                for k in range(CH):
                    t = ci * CH + k
                    # PE: scan matmul first, then this step's share of z MMs
                    nc.tensor.matmul(c_all[:, t:t + 1], mneg[:], partial[:],
                                     start=False, stop=(t == T - 1),
                                     skip_group_check=True)
                    if zg_cur is not None:
                        z_mms(zg_cur, k)
                    # ACT: (1+a)*s -> PSUM (feeds this step's STT)
                    sA = apsp.tile([128, 64], dt.float32, tag="sA")
                    nc.scalar.mul(sA[:], s_prev[:], float(a1_val))
                    # DVE: zero the decayed voltage where spiked (prev step)
                    nc.vector.copy_predicated(mvp[:], s_prev[:].bitcast(dt.int32),
                                              zero64[:])
                    # GPSIMD: q = mu + z_t
                    q = tmpp.tile([128, 64], dt.float32, tag="q")
                    nc.gpsimd.tensor_tensor(q[:], mu[:], zch[:, k * 64:(k + 1) * 64],
                                            mybir.AluOpType.add)
                    # DVE: u = (q + c) + sA
                    nc.vector.scalar_tensor_tensor(
                        u[:], q[:], c_all[:, t:t + 1], sA[:],
                        mybir.AluOpType.add, mybir.AluOpType.add)
                    # ACT: mu for next step
                    mu = tmpp.tile([128, 64], dt.float32, tag="mu")
                    nc.scalar.mul(mu[:], u[:], 0.75)
                    # DVE: v = mv + u   (mv = zeroed 0.9*v_prev)
                    vp = tmpp.tile([128, 64], dt.float32, tag="vp")
                    nc.vector.tensor_tensor(vp[:], mvp[:], u[:], mybir.AluOpType.add)
                    # ACT: decayed voltage for next step (reset applied next step)
                    mvp = tmpp.tile([128, 64], dt.float32, tag="mvp")
                    nc.scalar.mul(mvp[:], vp[:], 0.9)
                    # DVE: spikes + per-partition count
                    partial = tmpp.tile([128, 1], dt.bfloat16, tag="partial")
                    s_sl = sch[:, k * 64:(k + 1) * 64]
                    nc.vector.tensor_scalar(s_sl, vp[:], 1.0, 0.0,
                                            mybir.AluOpType.is_ge,
                                            mybir.AluOpType.add,
                                            accum_out=partial[:])
                    s_prev = s_sl
                if zg_cur is not None:
                    z_stores(*zwork[ci], zg_cur)
                nc.sync.dma_start(
                    sout_re[:, ci * CH:(ci + 1) * CH, :],
                    sch[:].rearrange("p (k e) -> p k e", e=64))
    nc.compile()
    return nc


def _round_mant(a, keep=11):
    """Round f32 to `keep` explicit mantissa bits (round-to-nearest)."""
    bits = a.view(np.uint32)
    shift = 23 - keep
    half = np.uint32(1 << (shift - 1))
    mask = np.uint32((0xFFFFFFFF << shift) & 0xFFFFFFFF)
    return ((bits + half) & mask).view(np.float32)


def _split(a):
    hi = _round_mant(np.ascontiguousarray(a, np.float32))
    lo = _round_mant((a.astype(np.float64) - hi).astype(np.float32))
    return hi, lo


def kernel(x, weight, self_excitation):
    import ml_dtypes
    x = np.asarray(x, dtype=np.float32)
    weight = np.asarray(weight, dtype=np.float32)
    a = np.float32(np.clip(np.asarray(self_excitation, np.float32), 0.0, 1.0)[0])
    A1 = np.float32(np.float32(1.0) + a)

    key = float(A1)
    if _cache.get("key") != key:
        _cache["nc"] = _build(key)
        _cache["key"] = key
    nc = _cache["nc"]

    wT = np.ascontiguousarray(weight.T)                     # [C, N]
    S = np.float32(64.0)
    wh, wl = _split(wT)
    wh16 = (wh / S).astype(np.float16)
    wl16 = (wl * S).astype(np.float16)
    xh, xl = _split(x)
    xh16 = (xh / S).astype(np.float16)
    xl16 = (xl * S).astype(np.float16)
    del xl
    blk = np.repeat(np.arange(8), 16)
    mneg = np.where(blk[:, None] == blk[None, :], np.float32(-1.0),
                    np.float32(0.0)).astype(ml_dtypes.bfloat16)
    rbrow = np.full((1, 128), RB, ml_dtypes.bfloat16)

    in_maps = []
    for c in range(NCORES):
        in_maps.append({
            "xh": xh[c * BL:(c + 1) * BL],
            "xh16": xh16[c * BL:(c + 1) * BL],
            "xl16": xl16[c * BL:(c + 1) * BL],
            "wh": wh,
            "wh16": wh16,
            "wl16": wl16,
            "mneg": mneg,
            "rbrow": rbrow,
        })
    global _last_in_maps
    _last_in_maps = in_maps
    res = run_bass_kernel_spmd(nc, in_maps, core_ids=list(range(NCORES)))

    out = np.zeros((B, N, T), np.float32)
    for c in range(NCORES):
        g = res.results[c]["sout"].reshape(T, BL, N)        # [t, b, n]
        out[c * BL:(c + 1) * BL, :, 1:] = g[:T - 1].transpose(1, 2, 0)
    return out


# revision 20
# speedup vs baseline: 1.2869x; 1.0594x over previous
"""AbstractKWTA kernel for 8 Trainium2 NeuronCores.

Model (per batch b, all in f32):
    z = weight @ x[b]                      # [N=1024, T=512], C=2048 contraction
    recurrent scan over T:
        tot  = sum_n s
        u    = 0.75*u + z_t + (1+a)*s + (RBIAS - tot)
        v    = 0.9*v + u
        s    = (v >= 1);  v = v * (1 - s)
    out[..., t+1] = s_t  (one-step delay, out[..., 0] = 0)

Sharding: data-parallel over batch B=64 -> 8 cores x 8 batches.

z matmul: split-fp32r — operands split hi/lo at 12 significant bits (fp32r
products are exact at that width); z = Xh'Wh + Xh'Wl + Xl'Wh at ~1.3 cyc/row
vs fp32's 4, fp32-class accuracy (err ~2e-7). Stationary is the x tile so z
lands in [t, n] layout, staged through DRAM.

Scan: [128 partitions = (batch, neuron-group), 64 free] tiles, one step every
~1.6us. The per-step critical cycle is kept entirely on the Vector engine:
cpred(reset mv) -> u(STT) -> v(TT) -> sge(+accumulate). The spike total
reaches u via a tiny bf16 PE matmul accumulating -tot onto an RBIAS-prefilled
PSUM column; (1+a)*s comes from ScalarE into PSUM. The voltage reset is
applied to the *decayed* copy mv = 0.9*v (exact: v*(1-s) then decay equals
decay then zero-where-spiked), keeping it off the critical path.

The z-production matmuls for t-blocks 1..3 are interleaved 6-per-step into
the scan of the preceding block, filling the PE's idle time, so only block 0
is produced up front. Rounding orders are chosen so the spike decisions
reproduce the f32 reference bit-exactly (verified: 0/33.5M mismatches).
"""

import numpy as np

import concourse.bacc as bacc
import concourse.tile as tile
from concourse import mybir
from concourse.bass_utils import run_bass_kernel_spmd
from concourse._compat import get_trn_type

dt = mybir.dt

B, C, T, N = 64, 2048, 512, 1024
NCORES = 8
BL = B // NCORES          # batches per core
RB = np.float32(1.0 * (N - 200) / N)   # 0.8046875: exact in f32 and bf16
CH = 16                   # scan steps per chunk; also matmul k-tiles per chunk
KC = C // 128             # 16 contraction tiles
NTCB = T // 128           # 4 t-blocks

_cache = {}


def _build(a1_val: float):
    nc = bacc.Bacc(get_trn_type() or "TRN2", target_bir_lowering=False,
                   debug=False, num_devices=NCORES)
    xa_in = nc.dram_tensor("xa", [BL, C, T], dt.float16, kind="ExternalInput").ap()
    xas_in = nc.dram_tensor("xas", [BL, C, T], dt.float16, kind="ExternalInput").ap()
    xb_in = nc.dram_tensor("xb", [BL, C, T], dt.float16, kind="ExternalInput").ap()
    wa_in = nc.dram_tensor("wa", [C, N], dt.float16, kind="ExternalInput").ap()
    was_in = nc.dram_tensor("was", [C, N], dt.float16, kind="ExternalInput").ap()
    wb_in = nc.dram_tensor("wb", [C, N], dt.float16, kind="ExternalInput").ap()
    mneg_in = nc.dram_tensor("mneg", [128, 128], dt.bfloat16, kind="ExternalInput").ap()
    rbrow_in = nc.dram_tensor("rbrow", [1, 128], dt.bfloat16, kind="ExternalInput").ap()
    sout = nc.dram_tensor("sout", [T, BL * N], dt.float32, kind="ExternalOutput").ap()

    with tile.TileContext(nc) as tc:
        with tc.tile_pool(name="const", bufs=1) as constp, \
             tc.tile_pool(name="wsb", bufs=1) as wsbp, \
             tc.tile_pool(name="xsb", bufs=2) as xsbp, \
             tc.tile_pool(name="zps", bufs=2, space="PSUM") as zpsp, \
             tc.tile_pool(name="zstg", bufs=4) as zstgp, \
             tc.tile_pool(name="zdram", bufs=1, space="DRAM") as zdramp, \
             tc.tile_pool(name="state", bufs=1) as statep, \
             tc.tile_pool(name="zch", bufs=3) as zchp, \
             tc.tile_pool(name="sch", bufs=3) as schp, \
             tc.tile_pool(name="tmp", bufs=4) as tmpp, \
             tc.tile_pool(name="cps", bufs=1, space="PSUM") as cpsp, \
             tc.tile_pool(name="aps", bufs=2, space="PSUM") as apsp:

            mneg = constp.tile([128, 128], dt.bfloat16, tag="mneg")
            nc.sync.dma_start(mneg[:], mneg_in[:])
            rbrow = constp.tile([1, 128], dt.bfloat16, tag="rbrow")
            nc.sync.dma_start(rbrow[:], rbrow_in[:])
            onec = constp.tile([1, T], dt.bfloat16, tag="onec")
            nc.vector.memset(onec[:], 1.0)
            zero64 = constp.tile([128, 64], dt.float32, tag="zero64")
            nc.vector.memset(zero64[:], 0.0)

            wa_sb = wsbp.tile([128, KC * N], dt.float16, tag="wa")
            was_sb = wsbp.tile([128, KC * N], dt.float16, tag="was")
            wb_sb = wsbp.tile([128, KC * N], dt.float16, tag="wb")
            for k in range(KC):
                nc.sync.dma_start(wa_sb[:, k * N:(k + 1) * N],
                                  wa_in[k * 128:(k + 1) * 128, :])
                nc.sync.dma_start(was_sb[:, k * N:(k + 1) * N],
                                  was_in[k * 128:(k + 1) * 128, :])
                nc.sync.dma_start(wb_sb[:, k * N:(k + 1) * N],
                                  wb_in[k * 128:(k + 1) * 128, :])

            zscr = zdramp.tile([T, BL * N], dt.float32, tag="zscr")
            xa_re = xa_in.rearrange("b (k p) t -> b p k t", p=128)
            xas_re = xas_in.rearrange("b (k p) t -> b p k t", p=128)
            xb_re = xb_in.rearrange("b (k p) t -> b p k t", p=128)

            def z_loads(tcb, b):
                xs = []
                for nm, re_ap in (("xa", xa_re), ("xas", xas_re), ("xb", xb_re)):
                    t = xsbp.tile([128, KC * 128], dt.float16, tag=nm)
                    nc.sync.dma_start(
                        t[:].rearrange("p (k t) -> p k t", t=128),
                        re_ap[b, :, :, tcb * 128:(tcb + 1) * 128])
                    xs.append(t)
                zp0 = zpsp.tile([128, 512], dt.float32, tag="zp0")
                zp1 = zpsp.tile([128, 512], dt.float32, tag="zp1")
                return tuple(xs), (zp0, zp1)

            def z_mms(g, k):
                (xa_sb, xas_sb, xb_sb), zp = g
                xa_k = xa_sb[:, k * 128:(k + 1) * 128]
                xas_k = xas_sb[:, k * 128:(k + 1) * 128]
                xb_k = xb_sb[:, k * 128:(k + 1) * 128]

                def wslc(w, nc2):
                    return w[:, k * N + nc2 * 512:k * N + nc2 * 512 + 512]

                # each stationary serves both n-halves consecutively;
                # prescales cancel within each product pair
                nc.tensor.matmul(zp[0][:], xa_k, wslc(wa_sb, 0),
                                 start=(k == 0), stop=False, skip_group_check=True)
                nc.tensor.matmul(zp[1][:], xa_k, wslc(wa_sb, 1),
                                 start=(k == 0), stop=False, skip_group_check=True)
                nc.tensor.matmul(zp[0][:], xas_k, wslc(wb_sb, 0),
                                 start=False, stop=False, skip_group_check=True)
                nc.tensor.matmul(zp[1][:], xas_k, wslc(wb_sb, 1),
                                 start=False, stop=False, skip_group_check=True)
                nc.tensor.matmul(zp[0][:], xb_k, wslc(was_sb, 0),
                                 start=False, stop=(k == KC - 1), skip_group_check=True)
                nc.tensor.matmul(zp[1][:], xb_k, wslc(was_sb, 1),
                                 start=False, stop=(k == KC - 1), skip_group_check=True)

            def z_stores(tcb, b, g):
                _, zp = g
                for nc2 in range(2):
                    zs = zstgp.tile([128, 512], dt.float32, tag="zs")
                    nc.scalar.copy(zs[:], zp[nc2][:])
                    nc.sync.dma_start(
                        zscr[tcb * 128:(tcb + 1) * 128,
                             b * N + nc2 * 512:b * N + nc2 * 512 + 512],
                        zs[:])

            # ---- t-block 0 (plus block 1's first group) produced up front,
            # with x loads prefetched one group ahead ----
            head = [(0, b) for b in range(BL)] + [(1, 0)]
            gh = z_loads(*head[0])
            for i, (tcb, b) in enumerate(head):
                g = gh
                if i + 1 < len(head):
                    gh = z_loads(*head[i + 1])
                for k in range(KC):
                    z_mms(g, k)
                z_stores(tcb, b, g)

            # remaining groups, one per scan chunk (one chunk of slack)
            zwork = [(tcb, b) for tcb in range(1, NTCB) for b in range(BL)][1:]

            # ---- scan state ----
            u = statep.tile([128, 64], dt.float32, tag="u")
            nc.vector.memset(u[:], 0.0)
            s_prev = statep.tile([128, 64], dt.float32, tag="sinit")
            nc.vector.memset(s_prev[:], 0.0)
            partial = statep.tile([128, 1], dt.bfloat16, tag="pinit")
            nc.vector.memset(partial[:], 0.0)
            mvp = statep.tile([128, 64], dt.float32, tag="mvinit")
            nc.vector.memset(mvp[:], 0.0)
            mu = tmpp.tile([128, 64], dt.float32, tag="mu")
            nc.scalar.mul(mu[:], u[:], 0.75)

            c_all = cpsp.tile([128, T], dt.float32, tag="call")
            nc.tensor.matmul(c_all[:], rbrow[:], onec[:], start=True, stop=False,
                             skip_group_check=True)

            zscr_re = zscr.rearrange("t (p e) -> p t e", p=128)
            sout_re = sout.rearrange("t (p e) -> p t e", p=128)
            zg_handles = {}
            if zwork:
                zg_handles[0] = z_loads(*zwork[0])
            for ci in range(T // CH):
                if ci + 1 < len(zwork):
                    zg_handles[ci + 1] = z_loads(*zwork[ci + 1])
                zg_cur = zg_handles.pop(ci, None)

                zch = zchp.tile([128, CH * 64], dt.float32)
                nc.sync.dma_start(
                    zch[:].rearrange("p (k e) -> p k e", e=64),
                    zscr_re[:, ci * CH:(ci + 1) * CH, :])
                sch = schp.tile([128, CH * 64], dt.float32)
                for k in range(CH):
                    t = ci * CH + k
                    # PE: scan matmul first, then this step's share of z MMs
                    nc.tensor.matmul(c_all[:, t:t + 1], mneg[:], partial[:],
                                     start=False, stop=(t == T - 1),
                                     skip_group_check=True)
                    if zg_cur is not None:
                        z_mms(zg_cur, k)
                    # ACT: (1+a)*s -> PSUM (feeds this step's STT)
                    sA = apsp.tile([128, 64], dt.float32, tag="sA")
                    nc.scalar.mul(sA[:], s_prev[:], float(a1_val))
                    # DVE: zero the decayed voltage where spiked (prev step)
                    nc.vector.copy_predicated(mvp[:], s_prev[:].bitcast(dt.int32),
                                              zero64[:])
                    # GPSIMD: q = mu + z_t
                    q = tmpp.tile([128, 64], dt.float32, tag="q")
                    nc.gpsimd.tensor_tensor(q[:], mu[:], zch[:, k * 64:(k + 1) * 64],
                                            mybir.AluOpType.add)
                    # DVE: u = (q + c) + sA
                    nc.vector.scalar_tensor_tensor(
                        u[:], q[:], c_all[:, t:t + 1], sA[:],
                        mybir.AluOpType.add, mybir.AluOpType.add)
                    # ACT: mu for next step
                    mu = tmpp.tile([128, 64], dt.float32, tag="mu")
                    nc.scalar.mul(mu[:], u[:], 0.75)
                    # DVE: v = mv + u   (mv = zeroed 0.9*v_prev)
                    vp = tmpp.tile([128, 64], dt.float32, tag="vp")
                    nc.vector.tensor_tensor(vp[:], mvp[:], u[:], mybir.AluOpType.add)
                    # ACT: decayed voltage for next step (reset applied next step)
                    mvp = tmpp.tile([128, 64], dt.float32, tag="mvp")
                    nc.scalar.mul(mvp[:], vp[:], 0.9)
                    # DVE: spikes + per-partition count
                    partial = tmpp.tile([128, 1], dt.bfloat16, tag="partial")
                    s_sl = sch[:, k * 64:(k + 1) * 64]
                    nc.vector.tensor_scalar(s_sl, vp[:], 1.0, 0.0,
                                            mybir.AluOpType.is_ge,
                                            mybir.AluOpType.add,
                                            accum_out=partial[:])
                    s_prev = s_sl
                if zg_cur is not None:
                    z_stores(*zwork[ci], zg_cur)
                nc.sync.dma_start(
                    sout_re[:, ci * CH:(ci + 1) * CH, :],
                    sch[:].rearrange("p (k e) -> p k e", e=64))
    nc.compile()
    return nc


def _round_mant(a, keep=11):
    """Round f32 to `keep` explicit mantissa bits (round-to-nearest)."""
    bits = a.view(np.uint32)
    shift = 23 - keep
    half = np.uint32(1 << (shift - 1))
    mask = np.uint32((0xFFFFFFFF << shift) & 0xFFFFFFFF)
    return ((bits + half) & mask).view(np.float32)


def _split(a):
    """Split f32 into two 11-significant-bit parts (fp16-exact)."""
    hi = _round_mant(np.ascontiguousarray(a, np.float32), 10)
    lo = _round_mant((a.astype(np.float64) - hi).astype(np.float32), 10)
    return hi, lo


def kernel(x, weight, self_excitation):
    import ml_dtypes
    x = np.asarray(x, dtype=np.float32)
    weight = np.asarray(weight, dtype=np.float32)
    a = np.float32(np.clip(np.asarray(self_excitation, np.float32), 0.0, 1.0)[0])
    A1 = np.float32(np.float32(1.0) + a)

    key = float(A1)
    if _cache.get("key") != key:
        _cache["nc"] = _build(key)
        _cache["key"] = key
    nc = _cache["nc"]

    wT = np.ascontiguousarray(weight.T)                     # [C, N]
    S = np.float32(32.0)
    wa32, wb32 = _split(wT)
    wa = wa32.astype(np.float16)
    was = (wa32 / S).astype(np.float16)
    wb = (wb32 * S).astype(np.float16)
    xa32, xb32 = _split(x)
    xa = xa32.astype(np.float16)
    xas = (xa32 / S).astype(np.float16)
    xb = (xb32 * S).astype(np.float16)
    del xa32, xb32
    blk = np.repeat(np.arange(8), 16)
    mneg = np.where(blk[:, None] == blk[None, :], np.float32(-1.0),
                    np.float32(0.0)).astype(ml_dtypes.bfloat16)
    rbrow = np.full((1, 128), RB, ml_dtypes.bfloat16)

    in_maps = []
    for c in range(NCORES):
        in_maps.append({
            "xa": xa[c * BL:(c + 1) * BL],
            "xas": xas[c * BL:(c + 1) * BL],
            "xb": xb[c * BL:(c + 1) * BL],
            "wa": wa,
            "was": was,
            "wb": wb,
            "mneg": mneg,
            "rbrow": rbrow,
        })
    global _last_in_maps
    _last_in_maps = in_maps
    res = run_bass_kernel_spmd(nc, in_maps, core_ids=list(range(NCORES)))

    out = np.zeros((B, N, T), np.float32)
    for c in range(NCORES):
        g = res.results[c]["sout"].reshape(T, BL, N)        # [t, b, n]
        out[c * BL:(c + 1) * BL, :, 1:] = g[:T - 1].transpose(1, 2, 0)
    return out
